# revision 2
# baseline (speedup 1.0000x reference)
"""Causal self-attention (B=4, T=2048, C=1024, H=16) on 8 trn2 NeuronCores.

Sharding: hybrid batch x head-group. Core c = (b, g) with b = c//2, g = c%2
owns batch b and heads 8g..8g+7. Per-core work equals the pure head-parallel
split (same MACs) but per-core DMA drops ~5x: x slice in is 4MB (vs 32MB
replicated), partial out is 8MB (vs 32MB). The host sums the two partial
out-projections per batch (the "all-reduce") and transposes back.

Device layouts (per core, head-pair hp in 0..3 indexes 2 heads):
  xh     [NJ, 128, NCT, 512]  x[b] tiles: partition=c%128, ct=c//128, t
  qk_sb  [128=2h*64, {q,k}, hp, T]   from the QKV matmuls (d on partitions)
  v_sb   [128=T%128, NKT, 512]       V computed directly in [T, d] layout
                                     (stationary = x tile, moving = WvT)
  S^T    [tk, tq]   scores transposed; softmax row-sum over partitions rides
                    inside the PV matmuls via augmented stationaries
  outT   [1024, T]  partial output, summed+transposed on host

All matmul operands are float16: full TensorE rate (1 cyc/row) at ANY free
dim (unlike fp32r which needs >=256), halves DVE elementwise cost (2x_1p
mode), halves LDWEIGHTS cost (FWL), and halves DMA bytes. PSUM accumulation
stays fp32. Verified absmax rel err ~1e-3 (tolerance 2e-2).

Causal structure: tq blocks of 512 per (j, hp); key tiles of 128. Diagonal
key-tiles (m = i-4j >= 0) only touch queries >= 128m, so their score/exp/
mask/PV work is trimmed to the live query range [128m, 512) - saves ~25% of
attention PE/ACT/DVE cycles vs full rectangles.

The PV stationaries are vA=[v_h0*pad | pad], vB=[pad | v_h1*pad], so PSUM
rows 0:64 of pyA hold Y_h0 and rows 64:128 the pad-masked row-sum; pyB is
mirrored. A constant swap matmul (fp32r, free rate at N=512) aligns each
head's row-sum with its Y partitions for the normalize multiply.
"""

import numpy as np
from contextlib import ExitStack

import concourse.bass as bass
import concourse.bacc as bacc
import concourse.mybir as mybir
import concourse.tile as tile
from concourse import bass_utils

B, T, C = 4, 2048, 1024
H, D = 16, 64
NCORES = 8
GPC = 2                      # head-groups (cores) per batch
HPG = H // GPC               # heads per core = 8
NHP = HPG // 2               # head-pairs per core = 4
CPG = HPG * D                # y-channels per core = 512
TQB = 512                    # tq block (matmul free dim)
NJ = T // TQB                # 4 tq blocks
NKT = T // 128               # 16 tk tiles
NCT = C // 128               # 8 contraction tiles for projections
F16 = mybir.dt.float16
FP32 = mybir.dt.float32
FP32R = mybir.dt.float32r
AF = mybir.ActivationFunctionType
SCALE = 1.0 / np.sqrt(D)

_cached = {}

# build-time config knobs (A/B testing)
CFG = {
    "trim": True,            # trim diagonal tiles to live query range
    "out_copy": "vector",    # PSUM->SBUF engine for out-proj tiles
    "qk_bias": "scalar",     # engine for QKV bias-add + PSUM->SBUF move
    "spsum_bufs": 2,
    "qkps_bufs": 2,
    "ppool_bufs": 6,
}


def _emit(tc, nc, xh, wh, bqk, bvrep, padrep, wouth, outT, reps=1):
    ctx = ExitStack()
    with ctx:
        const = ctx.enter_context(tc.tile_pool(name="const", bufs=1))
        xpool = ctx.enter_context(tc.tile_pool(name="xpool", bufs=2))
        qkvpool = ctx.enter_context(tc.tile_pool(name="qkvpool", bufs=1))
        ppool = ctx.enter_context(tc.tile_pool(name="ppool", bufs=CFG["ppool_bufs"]))
        ypool = ctx.enter_context(tc.tile_pool(name="ypool", bufs=2))
        opool = ctx.enter_context(tc.tile_pool(name="opool", bufs=3))
        spsum = ctx.enter_context(tc.tile_pool(name="spsum", bufs=CFG["spsum_bufs"], space="PSUM"))
        accps = ctx.enter_context(tc.tile_pool(name="accps", bufs=1, space="PSUM"))
        qkps = ctx.enter_context(tc.tile_pool(name="qkps", bufs=CFG["qkps_bufs"], space="PSUM"))

        # ---- constants ----
        # swap matrix: mirrors partition halves (and scales by 1/64 to undo
        # the 64-fold replication summed by the swap matmul).
        swap_f32 = const.tile([128, 128], FP32)
        nc.vector.memset(swap_f32, 0.0)
        nc.vector.memset(swap_f32[0:64, 64:128], 1.0 / 64.0)
        nc.vector.memset(swap_f32[64:128, 0:64], 1.0 / 64.0)
        swapm = const.tile([128, 128], FP32R)
        nc.vector.tensor_copy(swapm, swap_f32)
        # 4 diagonal-block causal masks, each replicated for the 2 heads:
        # mask2[m][p, h*512 + q] = 1.0 if p <= q - 128*m else 0.0
        mask2 = []
        for m in range(4):
            mk = const.tile([128, 2 * TQB], F16, name=f"mask2_{m}")
            nc.gpsimd.memset(mk, 1.0)
            for h in range(2):
                nc.gpsimd.affine_select(
                    out=mk[:, h * TQB:(h + 1) * TQB],
                    in_=mk[:, h * TQB:(h + 1) * TQB],
                    compare_op=mybir.AluOpType.is_ge,
                    fill=0.0,
                    base=-128 * m,
                    pattern=[[1, TQB]],
                    channel_multiplier=-1,
                )
            mask2.append(mk)

        # weights + biases + pad
        w_sb = const.tile([128, NCT, 3 * CPG], F16)
        nc.sync.dma_start(w_sb, wh)
        wo_sb = const.tile([128, NHP, 2 * CPG], F16)
        nc.sync.dma_start(wo_sb, wouth)
        b_sb = const.tile([128, 2 * NHP], FP32)
        nc.sync.dma_start(b_sb, bqk)
        bv_sb = const.tile([128, CPG], FP32)
        nc.sync.dma_start(bv_sb, bvrep)
        pr_sb = const.tile([128, NKT, 64], F16)
        nc.sync.dma_start(pr_sb, padrep)

        for rep in range(reps):
            # ---- QKV projection ----
            qk_sb = qkvpool.tile([128, 2, NHP, T], F16, name=f"{rep}_qk", tag="qk")
            v_sb = qkvpool.tile([128, NKT, CPG], F16, name=f"{rep}_v", tag="v")
            for jj in range(NJ):
                x_sb = xpool.tile([128, NCT, TQB], F16, name=f"{rep}_x_{jj}",
                                  tag="x")
                nc.sync.dma_start(x_sb, xh[jj])
                for hp in range(NHP):
                    for qk in range(2):
                        ps = qkps.tile([128, TQB], FP32,
                                       name=f"{rep}_qkps_{jj}_{hp}_{qk}", tag="qk")
                        col = qk * CPG + hp * 128
                        for ct in range(NCT):
                            nc.tensor.matmul(
                                ps,
                                lhsT=w_sb[:, ct, col:col + 128],
                                rhs=x_sb[:, ct, :],
                                start=(ct == 0), stop=(ct == NCT - 1))
                        dst = qk_sb[:, qk, hp, jj * TQB:(jj + 1) * TQB]
                        bias = b_sb[:, qk * NHP + hp:qk * NHP + hp + 1]
                        if CFG["qk_bias"] == "scalar":
                            nc.scalar.activation(dst, ps, AF.Identity, bias=bias)
                        else:
                            nc.vector.tensor_scalar_add(dst, ps, bias)
                # V directly in [T, d] layout: stationary = x tile
                for tt in range(TQB // 128):
                    i = jj * (TQB // 128) + tt
                    vps = qkps.tile([128, CPG], FP32, name=f"{rep}_vps_{i}",
                                    tag="qk")
                    for ct in range(NCT):
                        nc.tensor.matmul(
                            vps,
                            lhsT=x_sb[:, ct, tt * 128:(tt + 1) * 128],
                            rhs=w_sb[:, ct, 2 * CPG:3 * CPG],
                            start=(ct == 0), stop=(ct == NCT - 1))
                    nc.vector.tensor_add(v_sb[:, i, :], vps, bv_sb)

            # ---- augmented PV stationaries per head-pair ----
            vAs, vBs = [], []
            for hp in range(NHP):
                vA = qkvpool.tile([128, NKT, 128], F16, name=f"{rep}_vA{hp}",
                                  tag=f"vA{hp}")
                vB = qkvpool.tile([128, NKT, 128], F16, name=f"{rep}_vB{hp}",
                                  tag=f"vB{hp}")
                nc.vector.tensor_mul(vA[:, :, 0:64],
                                     v_sb[:, :, hp * 128:hp * 128 + 64], pr_sb)
                nc.vector.tensor_copy(vA[:, :, 64:128], pr_sb)
                nc.vector.tensor_mul(vB[:, :, 64:128],
                                     v_sb[:, :, hp * 128 + 64:hp * 128 + 128],
                                     pr_sb)
                nc.vector.tensor_copy(vB[:, :, 0:64], pr_sb)
                vAs.append(vA)
                vBs.append(vB)

            # ---- attention + out-projection per tq block ----
            for j in range(NJ):
                y_sb = ypool.tile([128, NHP, TQB], F16, name=f"{rep}_y_{j}",
                                  tag="y")
                ntk = 4 * (j + 1)
                for hp in range(NHP):
                    pyA = accps.tile([128, TQB], FP32, name=f"{rep}_pyA_{j}_{hp}",
                                     tag="pyA")
                    pyB = accps.tile([128, TQB], FP32, name=f"{rep}_pyB_{j}_{hp}",
                                     tag="pyB")
                    for i in range(ntk):
                        m = i - 4 * j
                        off = 128 * m if (CFG["trim"] and m > 0) else 0
                        ps2 = spsum.tile([128, 2 * TQB], FP32,
                                         name=f"{rep}_s_{j}_{hp}_{i}", tag="s")
                        for h in range(2):
                            nc.tensor.matmul(
                                ps2[:, h * TQB + off:(h + 1) * TQB],
                                lhsT=qk_sb[h * 64:(h + 1) * 64, 1, hp,
                                           i * 128:(i + 1) * 128],
                                rhs=qk_sb[h * 64:(h + 1) * 64, 0, hp,
                                          j * TQB + off:(j + 1) * TQB],
                                start=True, stop=True)
                        p_sb = ppool.tile([128, 2 * TQB], F16,
                                          name=f"{rep}_p_{j}_{hp}_{i}", tag="p")
                        if off == 0:
                            nc.scalar.activation(p_sb, ps2, AF.Exp,
                                                 scale=float(SCALE))
                            if m >= 0:
                                nc.vector.tensor_mul(p_sb, p_sb, mask2[m])
                        else:
                            for h in range(2):
                                sl = slice(h * TQB + off, (h + 1) * TQB)
                                nc.scalar.activation(p_sb[:, sl], ps2[:, sl],
                                                     AF.Exp, scale=float(SCALE))
                                nc.vector.tensor_mul(p_sb[:, sl], p_sb[:, sl],
                                                     mask2[m][:, sl])
                        first, last = (i == 0), (i == ntk - 1)
                        # rows 0:64 <- Y_h0, rows 64:128 <- rowsum_h0
                        nc.tensor.matmul(pyA[:, off:TQB], lhsT=vAs[hp][:, i, :],
                                         rhs=p_sb[:, off:TQB],
                                         start=first, stop=last)
                        # rows 0:64 <- rowsum_h1, rows 64:128 <- Y_h1
                        nc.tensor.matmul(pyB[:, off:TQB], lhsT=vBs[hp][:, i, :],
                                         rhs=p_sb[:, TQB + off:2 * TQB],
                                         start=first, stop=last)

                    # assemble [rowsum_h1 | rowsum_h0], mirror halves, divide
                    rs_sb = ypool.tile([128, TQB], FP32R,
                                       name=f"{rep}_rs_{j}_{hp}", tag="rs")
                    nc.vector.tensor_copy(rs_sb[0:64, :], pyB[0:64, :])
                    nc.vector.tensor_copy(rs_sb[64:128, :], pyA[64:128, :])
                    prs = qkps.tile([128, TQB], FP32, name=f"{rep}_prs_{j}_{hp}",
                                    tag="qk")
                    nc.tensor.matmul(prs, lhsT=swapm, rhs=rs_sb, start=True,
                                     stop=True)
                    recip = ypool.tile([128, TQB], FP32,
                                       name=f"{rep}_rc_{j}_{hp}", tag="rc")
                    nc.vector.reciprocal(recip, prs)
                    nc.vector.tensor_mul(y_sb[0:64, hp, :], pyA[0:64, :],
                                         recip[0:64, :])
                    nc.vector.tensor_mul(y_sb[64:128, hp, :], pyB[64:128, :],
                                         recip[64:128, :])

                # ---- out projection for this tq block ----
                for ot in range(NCT):
                    po = qkps.tile([128, TQB], FP32, name=f"{rep}_po_{j}_{ot}",
                                   tag="qk")
                    for hp in range(NHP):
                        nc.tensor.matmul(
                            po, lhsT=wo_sb[:, hp, ot * 128:(ot + 1) * 128],
                            rhs=y_sb[:, hp, :],
                            start=(hp == 0), stop=(hp == NHP - 1))
                    o_sb = opool.tile([128, TQB], FP32, name=f"{rep}_o_{j}_{ot}",
                                      tag="o")
                    if CFG["out_copy"] == "vector":
                        nc.vector.tensor_copy(o_sb, po)
                    else:
                        nc.scalar.activation(o_sb, po, AF.Identity)
                    nc.sync.dma_start(
                        outT[ot * 128:(ot + 1) * 128,
                             j * TQB:(j + 1) * TQB],
                        o_sb)


def build(reps=1):
    nc = bacc.Bacc()
    xh = nc.dram_tensor("xh", [NJ, 128, NCT, TQB], F16, kind="ExternalInput")
    wh = nc.dram_tensor("wh", [128, NCT, 3 * CPG], F16, kind="ExternalInput")
    bqk = nc.dram_tensor("bqk", [128, 2 * NHP], FP32, kind="ExternalInput")
    bvrep = nc.dram_tensor("bvrep", [128, CPG], FP32, kind="ExternalInput")
    padrep = nc.dram_tensor("padrep", [128, NKT, 64], F16, kind="ExternalInput")
    wouth = nc.dram_tensor("wouth", [128, NHP, 2 * CPG], F16,
                           kind="ExternalInput")
    outT = nc.dram_tensor("outT", [2 * CPG, T], FP32, kind="ExternalOutput")
    with tile.TileContext(nc) as tc:
        _emit(tc, nc, xh.ap(), wh.ap(), bqk.ap(), bvrep.ap(), padrep.ap(),
              wouth.ap(), outT.ap(), reps=reps)
    nc.compile()
    return nc


def make_in_maps(x, attention_mask, Wqkv, bqkv, Wout):
    x = np.asarray(x)
    attention_mask = np.asarray(attention_mask)
    Wqkv = np.asarray(Wqkv)
    bqkv = np.asarray(bqkv)
    Wout = np.asarray(Wout)
    in_maps = []
    for c in range(NCORES):
        b, g = divmod(c, GPC)
        # x[b] tiles: xh[jj, p, ct, t] = x[b, jj*512 + t, ct*128 + p]
        xh = np.ascontiguousarray(
            x[b].astype(np.float16).reshape(NJ, TQB, NCT, 128)
            .transpose(0, 3, 2, 1))
        rows = np.r_[g * CPG:(g + 1) * CPG,
                     C + g * CPG:C + (g + 1) * CPG,
                     2 * C + g * CPG:2 * C + (g + 1) * CPG]
        # wh[p, ct, col] = Wqkv[rows[col], ct*128 + p]
        wh = np.ascontiguousarray(
            Wqkv[rows, :].T.astype(np.float16)
            .reshape(NCT, 128, 3 * CPG).transpose(1, 0, 2))
        bq = bqkv[g * CPG:(g + 1) * CPG].astype(np.float32).reshape(NHP, 128)
        bk = bqkv[C + g * CPG:C + (g + 1) * CPG].astype(np.float32).reshape(
            NHP, 128)
        bqk = np.ascontiguousarray(
            np.concatenate([bq, bk], 0).T)           # [128, 8]
        bv = bqkv[2 * C + g * CPG:2 * C + (g + 1) * CPG].astype(np.float32)
        bvrep = np.ascontiguousarray(np.broadcast_to(bv, (128, CPG)))
        pad = attention_mask[b].astype(np.float16)   # [T]
        padrep = np.ascontiguousarray(np.broadcast_to(
            pad.reshape(NKT, 128, 1), (NKT, 128, 64)).transpose(1, 0, 2))
        # wouth[p, hp, o] = Wout[o, g*CPG + hp*128 + p]
        wouth = np.ascontiguousarray(
            Wout[:, g * CPG:(g + 1) * CPG].T.astype(np.float16)
            .reshape(NHP, 128, C).transpose(1, 0, 2))
        in_maps.append({"xh": xh, "wh": wh, "bqk": bqk, "bvrep": bvrep,
                        "padrep": padrep, "wouth": wouth})
    return in_maps


def kernel(x, attention_mask, Wqkv, bqkv, Wout, _trace=False):
    if "nc" not in _cached:
        _cached["nc"] = build()
    nc = _cached["nc"]
    in_maps = make_in_maps(x, attention_mask, Wqkv, bqkv, Wout)
    res = bass_utils.run_bass_kernel_spmd(
        nc, in_maps, core_ids=list(range(NCORES)), trace=_trace)
    out = np.empty((B, T, C), np.float32)
    for b in range(B):
        acc = res.results[GPC * b]["outT"].astype(np.float32)
        for g in range(1, GPC):
            acc += res.results[GPC * b + g]["outT"]
        out[b] = acc.T
    if _trace:
        _cached["last_result"] = res
    return out


# revision 5
# speedup vs baseline: 1.9552x; 1.9552x over previous
"""Causal self-attention (B=4, T=2048, C=1024, H=16) on 8 trn2 NeuronCores.

Sharding: hybrid batch x head-group. Core c = (b, g) with b = c//2, g = c%2
owns batch b and heads 8g..8g+7. Per-core work equals the pure head-parallel
split (same MACs) but per-core DMA drops ~5x: x slice in is 4MB (vs 32MB
replicated), partial out is 8MB (vs 32MB). The host sums the two partial
out-projections per batch (the "all-reduce") and transposes back.

Device layouts (per core, head-pair hp in 0..3 indexes 2 heads):
  xh     [NJ, 128, NCT, 512]  x[b] tiles: partition=c%128, ct=c//128, t
  qk_sb  [128=2h*64, {q,k}, hp, T]   from the QKV matmuls (d on partitions)
  v_sb   [128=T%128, NKT, 512]       V computed directly in [T, d] layout
                                     (stationary = x tile, moving = WvT)
  S^T    [tk, tq]   scores transposed; softmax row-sum over partitions rides
                    inside the PV matmuls via augmented stationaries
  outT   [1024, T]  partial output, summed+transposed on host

All matmul operands are float16: full TensorE rate (1 cyc/row) at ANY free
dim (unlike fp32r which needs >=256), halves DVE elementwise cost (2x_1p
mode), halves LDWEIGHTS cost (FWL), and halves DMA bytes. PSUM accumulation
stays fp32. Verified absmax rel err ~1e-3 (tolerance 2e-2).

Causal structure: tq blocks of 512 per (j, hp); key tiles of 128. Diagonal
key-tiles (m = i-4j >= 0) only touch queries >= 128m, so their score/exp/
mask/PV work is trimmed to the live query range [128m, 512) - saves ~25% of
attention PE/ACT/DVE cycles vs full rectangles.

The PV stationaries are vA=[v_h0*pad | pad], vB=[pad | v_h1*pad], so PSUM
rows 0:64 of pyA hold Y_h0 and rows 64:128 the pad-masked row-sum; pyB is
mirrored. A constant swap matmul (fp32r, free rate at N=512) aligns each
head's row-sum with its Y partitions for the normalize multiply.
"""

import numpy as np
from contextlib import ExitStack

import concourse.bass as bass
import concourse.bacc as bacc
import concourse.mybir as mybir
import concourse.tile as tile
from concourse import bass_utils

B, T, C = 4, 2048, 1024
H, D = 16, 64
NCORES = 8
GPC = 2                      # head-groups (cores) per batch
HPG = H // GPC               # heads per core = 8
NHP = HPG // 2               # head-pairs per core = 4
CPG = HPG * D                # y-channels per core = 512
TQB = 512                    # tq block (matmul free dim)
NJ = T // TQB                # 4 tq blocks
NKT = T // 128               # 16 tk tiles
NCT = C // 128               # 8 contraction tiles for projections
F16 = mybir.dt.float16
FP32 = mybir.dt.float32
FP32R = mybir.dt.float32r
AF = mybir.ActivationFunctionType
SCALE = 1.0 / np.sqrt(D)

_cached = {}

# build-time config knobs (A/B testing)
CFG = {
    "trim": True,            # trim diagonal tiles to live query range
    "out_copy": "vector",    # PSUM->SBUF engine for out-proj tiles
    "qk_bias": "scalar",     # engine for QKV bias-add + PSUM->SBUF move
    "spsum_bufs": 2,
    "qkps_bufs": 2,
    "ppool_bufs": 6,
}


def _emit(tc, nc, xh, wh, bqk, bvrep, padrep, wouth, outT, reps=1):
    ctx = ExitStack()
    with ctx:
        const = ctx.enter_context(tc.tile_pool(name="const", bufs=1))
        xpool = ctx.enter_context(tc.tile_pool(name="xpool", bufs=2))
        qkvpool = ctx.enter_context(tc.tile_pool(name="qkvpool", bufs=1))
        ppool = ctx.enter_context(tc.tile_pool(name="ppool", bufs=CFG["ppool_bufs"]))
        ypool = ctx.enter_context(tc.tile_pool(name="ypool", bufs=2))
        opool = ctx.enter_context(tc.tile_pool(name="opool", bufs=3))
        spsum = ctx.enter_context(tc.tile_pool(name="spsum", bufs=CFG["spsum_bufs"], space="PSUM"))
        accps = ctx.enter_context(tc.tile_pool(name="accps", bufs=1, space="PSUM"))
        qkps = ctx.enter_context(tc.tile_pool(name="qkps", bufs=CFG["qkps_bufs"], space="PSUM"))

        # ---- constants ----
        # swap matrix: mirrors partition halves (and scales by 1/64 to undo
        # the 64-fold replication summed by the swap matmul). fp16 so the
        # two 64x64 swap matmuls can col-tile (fp32r cannot) and run on
        # independent PE tiles.
        swapm = const.tile([128, 128], F16)
        nc.vector.memset(swapm, 0.0)
        nc.vector.memset(swapm[0:64, 64:128], 1.0 / 64.0)
        nc.vector.memset(swapm[64:128, 0:64], 1.0 / 64.0)
        # 4 diagonal-block causal masks, each replicated for the 2 heads:
        # mask2[m][p, h*512 + q] = 1.0 if p <= q - 128*m else 0.0
        mask2 = []
        for m in range(4):
            mk = const.tile([128, 2 * TQB], F16, name=f"mask2_{m}")
            nc.gpsimd.memset(mk, 1.0)
            for h in range(2):
                nc.gpsimd.affine_select(
                    out=mk[:, h * TQB:(h + 1) * TQB],
                    in_=mk[:, h * TQB:(h + 1) * TQB],
                    compare_op=mybir.AluOpType.is_ge,
                    fill=0.0,
                    base=-128 * m,
                    pattern=[[1, TQB]],
                    channel_multiplier=-1,
                )
            mask2.append(mk)

        # weights + biases + pad
        w_sb = const.tile([128, NCT, 3 * CPG], F16)
        nc.sync.dma_start(w_sb, wh)
        wo_sb = const.tile([128, NHP, 2 * CPG], F16)
        nc.sync.dma_start(wo_sb, wouth)
        b_sb = const.tile([128, 2 * NHP], FP32)
        nc.sync.dma_start(b_sb, bqk)
        bv_sb = const.tile([128, CPG], FP32)
        nc.sync.dma_start(bv_sb, bvrep)
        pr_sb = const.tile([128, NKT, 64], F16)
        nc.sync.dma_start(pr_sb, padrep)

        for rep in range(reps):
            # ---- QKV projection ----
            qk_sb = qkvpool.tile([128, 2, NHP, T], F16, name=f"{rep}_qk", tag="qk")
            v_sb = qkvpool.tile([128, NKT, CPG], F16, name=f"{rep}_v", tag="v")
            for jj in range(NJ):
                x_sb = xpool.tile([128, NCT, TQB], F16, name=f"{rep}_x_{jj}",
                                  tag="x")
                nc.sync.dma_start(x_sb, xh[jj])
                for hp in range(NHP):
                    for qk in range(2):
                        ps = qkps.tile([128, TQB], FP32,
                                       name=f"{rep}_qkps_{jj}_{hp}_{qk}", tag="qk")
                        col = qk * CPG + hp * 128
                        for ct in range(NCT):
                            nc.tensor.matmul(
                                ps,
                                lhsT=w_sb[:, ct, col:col + 128],
                                rhs=x_sb[:, ct, :],
                                start=(ct == 0), stop=(ct == NCT - 1))
                        dst = qk_sb[:, qk, hp, jj * TQB:(jj + 1) * TQB]
                        bias = b_sb[:, qk * NHP + hp:qk * NHP + hp + 1]
                        if CFG["qk_bias"] == "scalar":
                            nc.scalar.activation(dst, ps, AF.Identity, bias=bias)
                        else:
                            nc.vector.tensor_scalar_add(dst, ps, bias)
                # V directly in [T, d] layout: stationary = x tile
                for tt in range(TQB // 128):
                    i = jj * (TQB // 128) + tt
                    vps = qkps.tile([128, CPG], FP32, name=f"{rep}_vps_{i}",
                                    tag="qk")
                    for ct in range(NCT):
                        nc.tensor.matmul(
                            vps,
                            lhsT=x_sb[:, ct, tt * 128:(tt + 1) * 128],
                            rhs=w_sb[:, ct, 2 * CPG:3 * CPG],
                            start=(ct == 0), stop=(ct == NCT - 1))
                    nc.vector.tensor_add(v_sb[:, i, :], vps, bv_sb)

            # ---- augmented PV stationaries per head-pair ----
            vAs, vBs = [], []
            for hp in range(NHP):
                vA = qkvpool.tile([128, NKT, 128], F16, name=f"{rep}_vA{hp}",
                                  tag=f"vA{hp}")
                vB = qkvpool.tile([128, NKT, 128], F16, name=f"{rep}_vB{hp}",
                                  tag=f"vB{hp}")
                nc.vector.tensor_mul(vA[:, :, 0:64],
                                     v_sb[:, :, hp * 128:hp * 128 + 64], pr_sb)
                nc.vector.tensor_copy(vA[:, :, 64:128], pr_sb)
                nc.vector.tensor_mul(vB[:, :, 64:128],
                                     v_sb[:, :, hp * 128 + 64:hp * 128 + 128],
                                     pr_sb)
                nc.vector.tensor_copy(vB[:, :, 0:64], pr_sb)
                vAs.append(vA)
                vBs.append(vB)

            # ---- attention + out-projection per tq block ----
            for j in range(NJ):
                y_sb = ypool.tile([128, NHP, TQB], F16, name=f"{rep}_y_{j}",
                                  tag="y")
                ntk = 4 * (j + 1)
                for hp in range(NHP):
                    pyA = accps.tile([128, TQB], FP32, name=f"{rep}_pyA_{j}_{hp}",
                                     tag="pyA")
                    pyB = accps.tile([128, TQB], FP32, name=f"{rep}_pyB_{j}_{hp}",
                                     tag="pyB")
                    for i in range(ntk):
                        m = i - 4 * j
                        off = 128 * m if (CFG["trim"] and m > 0) else 0
                        ps2 = spsum.tile([128, 2 * TQB], FP32,
                                         name=f"{rep}_s_{j}_{hp}_{i}", tag="s")
                        for h in range(2):
                            nc.tensor.matmul(
                                ps2[:, h * TQB + off:(h + 1) * TQB],
                                lhsT=qk_sb[h * 64:(h + 1) * 64, 1, hp,
                                           i * 128:(i + 1) * 128],
                                rhs=qk_sb[h * 64:(h + 1) * 64, 0, hp,
                                          j * TQB + off:(j + 1) * TQB],
                                start=True, stop=True)
                        p_sb = ppool.tile([128, 2 * TQB], F16,
                                          name=f"{rep}_p_{j}_{hp}_{i}", tag="p")
                        if off == 0:
                            nc.scalar.activation(p_sb, ps2, AF.Exp,
                                                 scale=float(SCALE))
                            if m >= 0:
                                nc.vector.tensor_mul(p_sb, p_sb, mask2[m])
                        else:
                            for h in range(2):
                                sl = slice(h * TQB + off, (h + 1) * TQB)
                                nc.scalar.activation(p_sb[:, sl], ps2[:, sl],
                                                     AF.Exp, scale=float(SCALE))
                                nc.vector.tensor_mul(p_sb[:, sl], p_sb[:, sl],
                                                     mask2[m][:, sl])
                        first, last = (i == 0), (i == ntk - 1)
                        # rows 0:64 <- Y_h0, rows 64:128 <- rowsum_h0
                        nc.tensor.matmul(pyA[:, off:TQB], lhsT=vAs[hp][:, i, :],
                                         rhs=p_sb[:, off:TQB],
                                         start=first, stop=last)
                        # rows 0:64 <- rowsum_h1, rows 64:128 <- Y_h1
                        nc.tensor.matmul(pyB[:, off:TQB], lhsT=vBs[hp][:, i, :],
                                         rhs=p_sb[:, TQB + off:2 * TQB],
                                         start=first, stop=last)

                    # evacuate PSUM accumulators to SBUF right away so the
                    # next head-pair's PV can reuse the banks; the swap /
                    # reciprocal / normalize tail then runs off SBUF without
                    # stalling the PE.
                    ya_sb = ypool.tile([128, TQB], F16,
                                       name=f"{rep}_ya_{j}_{hp}", tag="ya")
                    yb_sb = ypool.tile([128, TQB], F16,
                                       name=f"{rep}_yb_{j}_{hp}", tag="yb")
                    nc.vector.tensor_copy(ya_sb, pyA)
                    nc.vector.tensor_copy(yb_sb, pyB)
                    # rows 0:64 <- rowsum_h1 (from yb), 64:128 <- rowsum_h0
                    # (from ya): mirror halves via the swap matmul, divide.
                    prs = qkps.tile([128, TQB], FP32, name=f"{rep}_prs_{j}_{hp}",
                                    tag="qk")
                    nc.tensor.matmul(prs[0:64, :], lhsT=swapm[64:128, 0:64],
                                     rhs=ya_sb[64:128, :], start=True, stop=True)
                    nc.tensor.matmul(prs[64:128, :], lhsT=swapm[0:64, 64:128],
                                     rhs=yb_sb[0:64, :], start=True, stop=True)
                    recip = ypool.tile([128, TQB], FP32,
                                       name=f"{rep}_rc_{j}_{hp}", tag="rc")
                    nc.vector.reciprocal(recip, prs)
                    nc.vector.tensor_mul(y_sb[0:64, hp, :], ya_sb[0:64, :],
                                         recip[0:64, :])
                    nc.vector.tensor_mul(y_sb[64:128, hp, :], yb_sb[64:128, :],
                                         recip[64:128, :])

                # ---- out projection for this tq block ----
                for ot in range(NCT):
                    po = qkps.tile([128, TQB], FP32, name=f"{rep}_po_{j}_{ot}",
                                   tag="qk")
                    for hp in range(NHP):
                        nc.tensor.matmul(
                            po, lhsT=wo_sb[:, hp, ot * 128:(ot + 1) * 128],
                            rhs=y_sb[:, hp, :],
                            start=(hp == 0), stop=(hp == NHP - 1))
                    o_sb = opool.tile([128, TQB], FP32, name=f"{rep}_o_{j}_{ot}",
                                      tag="o")
                    if CFG["out_copy"] == "vector":
                        nc.vector.tensor_copy(o_sb, po)
                    else:
                        nc.scalar.activation(o_sb, po, AF.Identity)
                    nc.sync.dma_start(
                        outT[ot * 128:(ot + 1) * 128,
                             j * TQB:(j + 1) * TQB],
                        o_sb)


def build(reps=1):
    nc = bacc.Bacc()
    xh = nc.dram_tensor("xh", [NJ, 128, NCT, TQB], F16, kind="ExternalInput")
    wh = nc.dram_tensor("wh", [128, NCT, 3 * CPG], F16, kind="ExternalInput")
    bqk = nc.dram_tensor("bqk", [128, 2 * NHP], FP32, kind="ExternalInput")
    bvrep = nc.dram_tensor("bvrep", [128, CPG], FP32, kind="ExternalInput")
    padrep = nc.dram_tensor("padrep", [128, NKT, 64], F16, kind="ExternalInput")
    wouth = nc.dram_tensor("wouth", [128, NHP, 2 * CPG], F16,
                           kind="ExternalInput")
    outT = nc.dram_tensor("outT", [2 * CPG, T], FP32, kind="ExternalOutput")
    with tile.TileContext(nc) as tc:
        _emit(tc, nc, xh.ap(), wh.ap(), bqk.ap(), bvrep.ap(), padrep.ap(),
              wouth.ap(), outT.ap(), reps=reps)
    nc.compile()
    return nc


def make_in_maps(x, attention_mask, Wqkv, bqkv, Wout):
    x = np.asarray(x)
    attention_mask = np.asarray(attention_mask)
    Wqkv = np.asarray(Wqkv)
    bqkv = np.asarray(bqkv)
    Wout = np.asarray(Wout)
    in_maps = []
    for c in range(NCORES):
        b, g = divmod(c, GPC)
        # x[b] tiles: xh[jj, p, ct, t] = x[b, jj*512 + t, ct*128 + p]
        xh = np.ascontiguousarray(
            x[b].astype(np.float16).reshape(NJ, TQB, NCT, 128)
            .transpose(0, 3, 2, 1))
        rows = np.r_[g * CPG:(g + 1) * CPG,
                     C + g * CPG:C + (g + 1) * CPG,
                     2 * C + g * CPG:2 * C + (g + 1) * CPG]
        # wh[p, ct, col] = Wqkv[rows[col], ct*128 + p]
        wh = np.ascontiguousarray(
            Wqkv[rows, :].T.astype(np.float16)
            .reshape(NCT, 128, 3 * CPG).transpose(1, 0, 2))
        bq = bqkv[g * CPG:(g + 1) * CPG].astype(np.float32).reshape(NHP, 128)
        bk = bqkv[C + g * CPG:C + (g + 1) * CPG].astype(np.float32).reshape(
            NHP, 128)
        bqk = np.ascontiguousarray(
            np.concatenate([bq, bk], 0).T)           # [128, 8]
        bv = bqkv[2 * C + g * CPG:2 * C + (g + 1) * CPG].astype(np.float32)
        bvrep = np.ascontiguousarray(np.broadcast_to(bv, (128, CPG)))
        pad = attention_mask[b].astype(np.float16)   # [T]
        padrep = np.ascontiguousarray(np.broadcast_to(
            pad.reshape(NKT, 128, 1), (NKT, 128, 64)).transpose(1, 0, 2))
        # wouth[p, hp, o] = Wout[o, g*CPG + hp*128 + p]
        wouth = np.ascontiguousarray(
            Wout[:, g * CPG:(g + 1) * CPG].T.astype(np.float16)
            .reshape(NHP, 128, C).transpose(1, 0, 2))
        in_maps.append({"xh": xh, "wh": wh, "bqk": bqk, "bvrep": bvrep,
                        "padrep": padrep, "wouth": wouth})
    return in_maps


def kernel(x, attention_mask, Wqkv, bqkv, Wout, _trace=False):
    if "nc" not in _cached:
        _cached["nc"] = build()
    nc = _cached["nc"]
    in_maps = make_in_maps(x, attention_mask, Wqkv, bqkv, Wout)
    res = bass_utils.run_bass_kernel_spmd(
        nc, in_maps, core_ids=list(range(NCORES)), trace=_trace)
    out = np.empty((B, T, C), np.float32)
    for b in range(B):
        acc = res.results[GPC * b]["outT"].astype(np.float32)
        for g in range(1, GPC):
            acc += res.results[GPC * b + g]["outT"]
        out[b] = acc.T
    if _trace:
        _cached["last_result"] = res
    return out


# revision 22
# speedup vs baseline: 2.4815x; 1.2692x over previous
"""Causal self-attention (B=4, T=2048, C=1024, H=16) on 8 trn2 NeuronCores.

Sharding: hybrid batch x head-group. Core c = (b, g) with b = c//2, g = c%2
owns batch b and heads 8g..8g+7. Per-core work equals the pure head-parallel
split (same MACs) but per-core DMA drops ~5x: x slice in is 4MB (vs 32MB
replicated), partial out is 8MB (vs 32MB). The host sums the two partial
out-projections per batch (the "all-reduce") and transposes back.

Device layouts (per core, head-pair hp in 0..3 indexes 2 heads):
  xh     [NJ, 128, NCT, 512]  x[b] tiles: partition=c%128, ct=c//128, t
  qk_sb  [128=2h*64, {q,k}, hp, T]   from the QKV matmuls (d on partitions)
  v_sb   [128=T%128, NKT, 512]       V computed directly in [T, d] layout
                                     (stationary = x tile, moving = WvT)
  S^T    [tk, tq]   scores transposed; softmax row-sum over partitions rides
                    inside the PV matmuls via augmented stationaries
  outT   [1024, T]  partial output, summed+transposed on host

All matmul operands are float16: full TensorE rate (1 cyc/row) at ANY free
dim (unlike fp32r which needs >=256), halves DVE elementwise cost (2x_1p
mode), halves LDWEIGHTS cost (FWL), and halves DMA bytes. PSUM accumulation
stays fp32. Verified absmax rel err ~1e-3 (tolerance 2e-2).

Causal structure: tq blocks of 512 per (j, hp); key tiles of 128. Diagonal
key-tiles (m = i-4j >= 0) only touch queries >= 128m, so their score/exp/
mask/PV work is trimmed to the live query range [128m, 512) - saves ~25% of
attention PE/ACT/DVE cycles vs full rectangles.

The PV stationaries are vA=[v_h0*pad | pad], vB=[pad | v_h1*pad], so PSUM
rows 0:64 of pyA hold Y_h0 and rows 64:128 the pad-masked row-sum; pyB is
mirrored. A constant swap matmul (fp32r, free rate at N=512) aligns each
head's row-sum with its Y partitions for the normalize multiply.
"""

import numpy as np
from contextlib import ExitStack

import concourse.bass as bass
import concourse.bacc as bacc
import concourse.mybir as mybir
import concourse.tile as tile
from concourse import bass_utils

B, T, C = 4, 2048, 1024
H, D = 16, 64
NCORES = 8
GPC = 2                      # head-groups (cores) per batch
HPG = H // GPC               # heads per core = 8
NHP = HPG // 2               # head-pairs per core = 4
CPG = HPG * D                # y-channels per core = 512
TQB = 512                    # tq block (matmul free dim)
NJ = T // TQB                # 4 tq blocks
NKT = T // 128               # 16 tk tiles
NCT = C // 128               # 8 contraction tiles for projections
F16 = mybir.dt.float16
FP32 = mybir.dt.float32
FP32R = mybir.dt.float32r
AF = mybir.ActivationFunctionType
SCALE = 1.0 / np.sqrt(D)

_cached = {}

# build-time config knobs (A/B testing)
CFG = {
    "trim": True,            # trim diagonal tiles to live query range
    "out_copy": "vector",    # PSUM->SBUF engine for out-proj tiles
    "qk_bias": "scalar",     # engine for QKV bias-add + PSUM->SBUF move
    "pipe": 2,               # PV lags scores by this many tiles (PE stays fed
                             # while ACT exp / DVE mask process each tile)
    "interleave": "fine",    # QKV block jj+1 vs attention block j: "coarse"
                             # emits it as one chunk before, "fine" sprinkles
                             # its matmul groups between attention tiles,
                             # False emits all QKV then all attention
    "defer_out": True,       # hold out-proj(j) until attention(j+1)
    "spsum_bufs": 2,
    "qkps_bufs": 2,
    "ppool_bufs": 8,
}


def _emit(tc, nc, xh, wh, bqk, bvrep, padrep, wouth, outT, reps=1):
    ctx = ExitStack()
    with ctx:
        const = ctx.enter_context(tc.tile_pool(name="const", bufs=1))
        xpool = ctx.enter_context(tc.tile_pool(name="xpool", bufs=2))
        qkvpool = ctx.enter_context(tc.tile_pool(name="qkvpool", bufs=1))
        ppool = ctx.enter_context(tc.tile_pool(name="ppool", bufs=CFG["ppool_bufs"]))
        ypool = ctx.enter_context(tc.tile_pool(name="ypool", bufs=2))
        opool = ctx.enter_context(tc.tile_pool(name="opool", bufs=3))
        spsum = ctx.enter_context(tc.tile_pool(name="spsum", bufs=CFG["spsum_bufs"], space="PSUM"))
        accps = ctx.enter_context(tc.tile_pool(name="accps", bufs=1, space="PSUM"))
        qkps = ctx.enter_context(tc.tile_pool(name="qkps", bufs=CFG["qkps_bufs"], space="PSUM"))

        # ---- constants ----
        # swap matrix: mirrors partition halves (and scales by 1/64 to undo
        # the 64-fold replication summed by the swap matmul). fp16 so the
        # two 64x64 swap matmuls can col-tile (fp32r cannot) and run on
        # independent PE tiles.
        swapm = const.tile([128, 128], F16)
        nc.vector.memset(swapm, 0.0)
        nc.vector.memset(swapm[0:64, 64:128], 1.0 / 64.0)
        nc.vector.memset(swapm[64:128, 0:64], 1.0 / 64.0)
        # 4 diagonal-block causal masks, each replicated for the 2 heads:
        # mask2[m][p, h*512 + q] = 1.0 if p <= q - 128*m else 0.0
        mask2 = []
        for m in range(4):
            mk = const.tile([128, 2 * TQB], F16, name=f"mask2_{m}")
            nc.gpsimd.memset(mk, 1.0)
            for h in range(2):
                nc.gpsimd.affine_select(
                    out=mk[:, h * TQB:(h + 1) * TQB],
                    in_=mk[:, h * TQB:(h + 1) * TQB],
                    compare_op=mybir.AluOpType.is_ge,
                    fill=0.0,
                    base=-128 * m,
                    pattern=[[1, TQB]],
                    channel_multiplier=-1,
                )
            mask2.append(mk)

        # weights + biases + pad. DMA order matters on the SP queue: the
        # first QKV matmul needs x tile 0 and w ct=0, so those go first
        # (per-ct chunks); bulky late-use tensors (wout, padrep) go last.
        x0_sb = xpool.tile([128, NCT, TQB], F16, name="0_x_0", tag="x")
        w_sb = const.tile([128, NCT, 3 * CPG], F16)
        b_sb = const.tile([128, 2 * NHP], FP32)
        bv_sb = const.tile([128, CPG], FP32)
        for ct in range(NCT):
            nc.sync.dma_start(x0_sb[:, ct, :], xh[0, :, ct, :])
            nc.sync.dma_start(w_sb[:, ct, :], wh[:, ct, :])
        nc.sync.dma_start(b_sb, bqk)
        nc.sync.dma_start(bv_sb, bvrep)
        pr_sb = const.tile([128, NKT, 64], F16)
        nc.sync.dma_start(pr_sb, padrep)
        wo_sb = const.tile([128, NHP, 2 * CPG], F16)
        nc.sync.dma_start(wo_sb, wouth)

        for rep in range(reps):
            qk_sb = qkvpool.tile([128, 2, NHP, T], F16, name=f"{rep}_qk", tag="qk")
            v_sb = qkvpool.tile([128, NKT, CPG], F16, name=f"{rep}_v", tag="v")
            vAs, vBs = [], []
            for hp in range(NHP):
                vAs.append(qkvpool.tile([128, NKT, 128], F16,
                                        name=f"{rep}_vA{hp}", tag=f"vA{hp}"))
                vBs.append(qkvpool.tile([128, NKT, 128], F16,
                                        name=f"{rep}_vB{hp}", tag=f"vB{hp}"))

            def qkv_groups(jj):
                """QKV projection for tq block jj as a list of emitter
                closures (one per matmul group) so the scheduler can
                sprinkle them between attention tiles."""
                hold = {}

                def g_x():
                    if rep == 0 and jj == 0:
                        hold["x"] = x0_sb
                    else:
                        x_sb = xpool.tile([128, NCT, TQB], F16,
                                          name=f"{rep}_x_{jj}", tag="x")
                        nc.sync.dma_start(x_sb, xh[jj])
                        hold["x"] = x_sb

                def g_qk(hp, qk):
                    def emit():
                        x_sb = hold["x"]
                        ps = qkps.tile([128, TQB], FP32,
                                       name=f"{rep}_qkps_{jj}_{hp}_{qk}",
                                       tag="qk")
                        col = qk * CPG + hp * 128
                        for ct in range(NCT):
                            nc.tensor.matmul(
                                ps,
                                lhsT=w_sb[:, ct, col:col + 128],
                                rhs=x_sb[:, ct, :],
                                start=(ct == 0), stop=(ct == NCT - 1))
                        dst = qk_sb[:, qk, hp, jj * TQB:(jj + 1) * TQB]
                        bias = b_sb[:, qk * NHP + hp:qk * NHP + hp + 1]
                        if CFG["qk_bias"] == "scalar":
                            nc.scalar.activation(dst, ps, AF.Identity,
                                                 bias=bias)
                        else:
                            nc.vector.tensor_scalar_add(dst, ps, bias)
                    return emit

                def g_v(tt):
                    def emit():
                        x_sb = hold["x"]
                        i = jj * (TQB // 128) + tt
                        vps = qkps.tile([128, CPG], FP32,
                                        name=f"{rep}_vps_{i}", tag="qk")
                        for ct in range(NCT):
                            nc.tensor.matmul(
                                vps,
                                lhsT=x_sb[:, ct, tt * 128:(tt + 1) * 128],
                                rhs=w_sb[:, ct, 2 * CPG:3 * CPG],
                                start=(ct == 0), stop=(ct == NCT - 1))
                        nc.vector.tensor_add(v_sb[:, i, :], vps, bv_sb)
                    return emit

                def g_vab():
                    ts = slice(jj * 4, (jj + 1) * 4)
                    for hp in range(NHP):
                        c0, c1 = hp * 128, hp * 128 + 64
                        nc.vector.tensor_mul(vAs[hp][:, ts, 0:64],
                                             v_sb[:, ts, c0:c0 + 64],
                                             pr_sb[:, ts, :])
                        nc.vector.tensor_copy(vAs[hp][:, ts, 64:128],
                                              pr_sb[:, ts, :])
                        nc.vector.tensor_mul(vBs[hp][:, ts, 64:128],
                                             v_sb[:, ts, c1:c1 + 64],
                                             pr_sb[:, ts, :])
                        nc.vector.tensor_copy(vBs[hp][:, ts, 0:64],
                                              pr_sb[:, ts, :])

                head = [g_x] + [g_qk(hp, qk) for hp in range(NHP)
                                for qk in range(2)]
                vtail = [g_v(tt) for tt in range(TQB // 128)] + [g_vab]
                return head, vtail

            def emit_qkv(jj):
                head, vtail = qkv_groups(jj)
                for g in head + vtail:
                    g()

            # ---- attention + out-projection, software-pipelined ----
            # Scores/exp/mask for tile i are emitted immediately; the PV
            # matmuls (which depend on exp+mask output) are deferred by
            # CFG["pipe"] tiles, and the per-head-pair normalize tail plus
            # the per-j out-projection are deferred past the next group's
            # first scores. This keeps the PE instruction queue free of
            # ops that would stall on ACT/DVE latency.
            pend = []

            def flush(n):
                while len(pend) > n:
                    pend.pop(0)()

            def mk_pv(hp, i, off, p_sb, pyA, pyB, first, last):
                def emit():
                    nc.tensor.matmul(pyA[:, off:TQB], lhsT=vAs[hp][:, i, :],
                                     rhs=p_sb[:, off:TQB],
                                     start=first, stop=last)
                    nc.tensor.matmul(pyB[:, off:TQB],
                                     lhsT=vBs[hp][:, i, :],
                                     rhs=p_sb[:, TQB + off:2 * TQB],
                                     start=first, stop=last)
                return emit

            def mk_tail(j, hp, pyA, pyB, y_sb):
                def emit():
                    # evacuate PSUM accumulators to SBUF right away so the
                    # next head-pair's PV can reuse the banks; the swap /
                    # reciprocal / normalize tail runs off SBUF.
                    ya_sb = ypool.tile([128, TQB], F16,
                                       name=f"{rep}_ya_{j}_{hp}", tag="ya")
                    yb_sb = ypool.tile([128, TQB], F16,
                                       name=f"{rep}_yb_{j}_{hp}", tag="yb")
                    nc.vector.tensor_copy(ya_sb, pyA)
                    nc.vector.tensor_copy(yb_sb, pyB)
                    # rows 0:64 <- rowsum_h0 (mirrored from ya's 64:128),
                    # rows 64:128 <- rowsum_h1 (mirrored from yb's 0:64)
                    prs = qkps.tile([128, TQB], FP32,
                                    name=f"{rep}_prs_{j}_{hp}", tag="qk")
                    nc.tensor.matmul(prs[0:64, :], lhsT=swapm[64:128, 0:64],
                                     rhs=ya_sb[64:128, :], start=True,
                                     stop=True)
                    nc.tensor.matmul(prs[64:128, :], lhsT=swapm[0:64, 64:128],
                                     rhs=yb_sb[0:64, :], start=True, stop=True)
                    recip = ypool.tile([128, TQB], FP32,
                                       name=f"{rep}_rc_{j}_{hp}", tag="rc")
                    nc.vector.reciprocal(recip, prs)
                    nc.vector.tensor_mul(y_sb[0:64, hp, :], ya_sb[0:64, :],
                                         recip[0:64, :])
                    nc.vector.tensor_mul(y_sb[64:128, hp, :], yb_sb[64:128, :],
                                         recip[64:128, :])
                return emit

            def mk_outproj(j, y_sb):
                def emit():
                    for ot in range(NCT):
                        po = qkps.tile([128, TQB], FP32,
                                       name=f"{rep}_po_{j}_{ot}", tag="qk")
                        for hp in range(NHP):
                            nc.tensor.matmul(
                                po,
                                lhsT=wo_sb[:, hp, ot * 128:(ot + 1) * 128],
                                rhs=y_sb[:, hp, :],
                                start=(hp == 0), stop=(hp == NHP - 1))
                        o_sb = opool.tile([128, TQB], F16,
                                          name=f"{rep}_o_{j}_{ot}", tag="o")
                        if CFG["out_copy"] == "vector":
                            nc.vector.tensor_copy(o_sb, po)
                        else:
                            nc.scalar.activation(o_sb, po, AF.Identity)
                        nc.sync.dma_start(
                            outT[ot * 128:(ot + 1) * 128,
                                 j * TQB:(j + 1) * TQB],
                            o_sb)
                return emit

            outps = {}

            def emit_attn(j, fillers=(), start_ops=(), mid_ops=(),
                          fstride=None):
                fillers = list(fillers)
                n_tiles = NHP * 4 * (j + 1)
                stride = fstride or (max(1, n_tiles // len(fillers))
                                     if fillers else 0)
                tile_no = 0
                for op in start_ops:
                    pend.append(op)
                y_sb = ypool.tile([128, NHP, TQB], F16, name=f"{rep}_y_{j}",
                                  tag="y")
                ntk = 4 * (j + 1)
                for hp in range(NHP):
                    if hp == 2:
                        for op in mid_ops:
                            pend.append(op)
                    pyA = accps.tile([128, TQB], FP32,
                                     name=f"{rep}_pyA_{j}_{hp}", tag="pyA")
                    pyB = accps.tile([128, TQB], FP32,
                                     name=f"{rep}_pyB_{j}_{hp}", tag="pyB")
                    for i in range(ntk):
                        m = i - 4 * j
                        off = 128 * m if (CFG["trim"] and m > 0) else 0
                        ps2 = spsum.tile([128, 2 * TQB], FP32,
                                         name=f"{rep}_s_{j}_{hp}_{i}", tag="s")
                        for h in range(2):
                            nc.tensor.matmul(
                                ps2[:, h * TQB + off:(h + 1) * TQB],
                                lhsT=qk_sb[h * 64:(h + 1) * 64, 1, hp,
                                           i * 128:(i + 1) * 128],
                                rhs=qk_sb[h * 64:(h + 1) * 64, 0, hp,
                                          j * TQB + off:(j + 1) * TQB],
                                start=True, stop=True)
                        p_sb = ppool.tile([128, 2 * TQB], F16,
                                          name=f"{rep}_p_{j}_{hp}_{i}", tag="p")
                        if off == 0:
                            nc.scalar.activation(p_sb, ps2, AF.Exp,
                                                 scale=float(SCALE))
                            if m >= 0:
                                nc.vector.tensor_mul(p_sb, p_sb, mask2[m])
                        else:
                            for h in range(2):
                                sl = slice(h * TQB + off, (h + 1) * TQB)
                                nc.scalar.activation(p_sb[:, sl], ps2[:, sl],
                                                     AF.Exp, scale=float(SCALE))
                                nc.vector.tensor_mul(p_sb[:, sl], p_sb[:, sl],
                                                     mask2[m][:, sl])
                        pend.append(mk_pv(hp, i, off, p_sb, pyA, pyB,
                                          i == 0, i == ntk - 1))
                        flush(CFG["pipe"])
                        tile_no += 1
                        if fillers and tile_no % stride == 0:
                            fillers.pop(0)()
                    pend.append(mk_tail(j, hp, pyA, pyB, y_sb))
                for g in fillers:
                    g()
                outps[j] = mk_outproj(j, y_sb)
                if not CFG["defer_out"]:
                    pend.append(outps.pop(j))

            # Interleave: attention block j needs only QKV blocks <= j, so
            # QKV block j+1 overlaps attention block j - its dense PE work
            # keeps the PE fed through the ACT-exp-bound attention phases.
            # "fine" sprinkles the QKV matmul groups between attention tiles.
            # The last attention block has no QKV left to overlap and its
            # exp load is the largest, so QKV(3)'s V-projection part and two
            # deferred out-projections are parked there as PE filler.
            if CFG["interleave"] == "fine" and CFG["defer_out"]:
                h1, v1 = qkv_groups(1)
                h2, v2 = qkv_groups(2)
                h3, v3 = qkv_groups(3)
                emit_qkv(0)
                emit_attn(0, h1 + v1)
                emit_attn(1, h2 + v2)
                emit_attn(2, h3, start_ops=[outps.pop(0)])
                emit_attn(3, v3, start_ops=[outps.pop(1)],
                          mid_ops=[outps.pop(2)], fstride=2)
                flush(0)
                outps.pop(3)()
            else:
                if CFG["interleave"] == "fine":
                    emit_qkv(0)
                    for j in range(NJ):
                        hv = qkv_groups(j + 1) if j + 1 < NJ else ([], [])
                        emit_attn(j, hv[0] + hv[1])
                elif CFG["interleave"]:
                    emit_qkv(0)
                    for j in range(NJ):
                        if j + 1 < NJ:
                            emit_qkv(j + 1)
                        emit_attn(j)
                else:
                    for jj in range(NJ):
                        emit_qkv(jj)
                    for j in range(NJ):
                        emit_attn(j)
                flush(0)
                for j in sorted(outps):
                    outps.pop(j)()


def build(reps=1):
    nc = bacc.Bacc()
    xh = nc.dram_tensor("xh", [NJ, 128, NCT, TQB], F16, kind="ExternalInput")
    wh = nc.dram_tensor("wh", [128, NCT, 3 * CPG], F16, kind="ExternalInput")
    bqk = nc.dram_tensor("bqk", [128, 2 * NHP], FP32, kind="ExternalInput")
    bvrep = nc.dram_tensor("bvrep", [128, CPG], FP32, kind="ExternalInput")
    padrep = nc.dram_tensor("padrep", [128, NKT, 64], F16, kind="ExternalInput")
    wouth = nc.dram_tensor("wouth", [128, NHP, 2 * CPG], F16,
                           kind="ExternalInput")
    outT = nc.dram_tensor("outT", [2 * CPG, T], F16, kind="ExternalOutput")
    with tile.TileContext(nc) as tc:
        _emit(tc, nc, xh.ap(), wh.ap(), bqk.ap(), bvrep.ap(), padrep.ap(),
              wouth.ap(), outT.ap(), reps=reps)
    nc.compile()
    return nc


def make_in_maps(x, attention_mask, Wqkv, bqkv, Wout):
    x = np.asarray(x)
    attention_mask = np.asarray(attention_mask)
    Wqkv = np.asarray(Wqkv)
    bqkv = np.asarray(bqkv)
    Wout = np.asarray(Wout)
    in_maps = []
    for c in range(NCORES):
        b, g = divmod(c, GPC)
        # x[b] tiles: xh[jj, p, ct, t] = x[b, jj*512 + t, ct*128 + p]
        xh = np.ascontiguousarray(
            x[b].astype(np.float16).reshape(NJ, TQB, NCT, 128)
            .transpose(0, 3, 2, 1))
        rows = np.r_[g * CPG:(g + 1) * CPG,
                     C + g * CPG:C + (g + 1) * CPG,
                     2 * C + g * CPG:2 * C + (g + 1) * CPG]
        # wh[p, ct, col] = Wqkv[rows[col], ct*128 + p]
        wh = np.ascontiguousarray(
            Wqkv[rows, :].T.astype(np.float16)
            .reshape(NCT, 128, 3 * CPG).transpose(1, 0, 2))
        bq = bqkv[g * CPG:(g + 1) * CPG].astype(np.float32).reshape(NHP, 128)
        bk = bqkv[C + g * CPG:C + (g + 1) * CPG].astype(np.float32).reshape(
            NHP, 128)
        bqk = np.ascontiguousarray(
            np.concatenate([bq, bk], 0).T)           # [128, 8]
        bv = bqkv[2 * C + g * CPG:2 * C + (g + 1) * CPG].astype(np.float32)
        bvrep = np.ascontiguousarray(np.broadcast_to(bv, (128, CPG)))
        pad = attention_mask[b].astype(np.float16)   # [T]
        padrep = np.ascontiguousarray(np.broadcast_to(
            pad.reshape(NKT, 128, 1), (NKT, 128, 64)).transpose(1, 0, 2))
        # wouth[p, hp, o] = Wout[o, g*CPG + hp*128 + p]
        wouth = np.ascontiguousarray(
            Wout[:, g * CPG:(g + 1) * CPG].T.astype(np.float16)
            .reshape(NHP, 128, C).transpose(1, 0, 2))
        in_maps.append({"xh": xh, "wh": wh, "bqk": bqk, "bvrep": bvrep,
                        "padrep": padrep, "wouth": wouth})
    return in_maps


def kernel(x, attention_mask, Wqkv, bqkv, Wout, _trace=False):
    if "nc" not in _cached:
        _cached["nc"] = build()
    nc = _cached["nc"]
    in_maps = make_in_maps(x, attention_mask, Wqkv, bqkv, Wout)
    res = bass_utils.run_bass_kernel_spmd(
        nc, in_maps, core_ids=list(range(NCORES)), trace=_trace)
    out = np.empty((B, T, C), np.float32)
    for b in range(B):
        acc = res.results[GPC * b]["outT"].astype(np.float32)
        for g in range(1, GPC):
            acc += res.results[GPC * b + g]["outT"]
        out[b] = acc.T
    if _trace:
        _cached["last_result"] = res
    return out
